# revision 1
# baseline (speedup 1.0000x reference)
"""GAE (generalized advantage estimation) Trainium2 kernel.

Problem: nn_CustomGAE — B=512, T=2048, D=64.
  value = obs @ W + b ; next_value = next_obs @ W + b
  td0 = reward + gamma*nd*next_value - value ; coef = gamma*lambda*nd
  A_t = td0_t + coef_t * A_{t+1}  (reverse scan over T, independent per trajectory)
  returns (advantage, value_target = advantage + value)

Sharding: pure data parallel over B across 8 cores (64 trajectories/core).

Per-core layout: the host pre-swizzles each 64-trajectory shard to
(half, batch)-major, so SBUF partition p = h*64 + b holds timesteps
t in [h*1024, (h+1)*1024) at a uniform DRAM stride — every streamed chunk is
one 128-partition dma_start with 16KB-contiguous per-partition runs.

The value-head matvec streams obs/next_obs in chunks: DVE does obs*W
(in-place) + both segmented reduces, GPSIMD does next_obs*W, so no single
engine exceeds the HBM DMA time. The backward recurrence runs as DVE
tensor_tensor_scan over reversed-stride APs (second half first, the
boundary value carried to the first half via a tiny SBUF->SBUF DMA).
"""

import sys

sys.path.insert(0, "/opt/trn_rl_repo")

from contextlib import ExitStack

import numpy as np

import concourse.bacc as bacc
import concourse.mybir as mybir
import concourse.tile as tile
from concourse.bass_utils import run_bass_kernel_spmd

GAMMA = 0.99
LMBDA = 0.95

B, T, D = 512, 2048, 64
NCORES = 8
BL = B // NCORES  # 64 trajectories per core
H = 2  # trajectory halves stacked on partitions -> 128 partitions
P = H * BL  # 128
F32 = mybir.dt.float32
U8 = mybir.dt.uint8

# Results of the last hardware run, for test harnesses.
LAST_RESULTS = None


def _build_iter(
    nc, opool, npool, ppool, dpool, w_t, b_t, bnd,
    obs_d, nobs_d, rw_d, dn_d, adv_d, tgt_d, tp, tc_sz, nchunk,
    dual_dma=False, nocompute=False, out_scalar=True,
):
    """One full pass: load inputs, matvec, scan, write outputs.

    Engine/ring discipline: ALL input streaming runs on the SP (sync) HWDGE
    ring with waits only on pool-slot availability, so it never stalls
    behind compute. Output + boundary DMAs go on the Activation ring.
    Tiles written early in an iteration but read late in the previous one
    (v_raw, nv_raw, rw_t, dn_t) come from a bufs=2 pool so back-to-back
    iterations don't serialize on WAW."""
    mult = mybir.AluOpType.mult
    add = mybir.AluOpType.add
    sub = mybir.AluOpType.subtract
    # second HWDGE engine (Activation) for the next_obs stream
    eng2 = nc.scalar if dual_dma else nc.sync
    oeng = nc.scalar if out_scalar else nc.sync

    v_raw = dpool.tile([P, tp], F32)  # obs @ W (no bias)
    nv_raw = dpool.tile([P, tp], F32)  # next_obs @ W (no bias)
    rw_t = dpool.tile([P, tp], F32)
    dn_t = dpool.tile([P, tp], U8)
    nc.sync.dma_start(rw_t[:], rw_d.ap())
    nc.sync.dma_start(dn_t[:], dn_d.ap())

    # done -> nd-derived factors, issued BEFORE the chunk stream so they
    # overlap it instead of sitting in the post-stream serial tail.
    ndf = dpool.tile([P, tp], F32)
    nc.vector.tensor_copy(ndf[:], dn_t[:])  # u8 -> f32
    g = dpool.tile([P, tp], F32)  # gamma * nd
    nc.scalar.activation(
        g[:], ndf[:], mybir.ActivationFunctionType.Copy, bias=GAMMA, scale=-GAMMA
    )
    coef = dpool.tile([P, tp], F32)  # gamma * lambda * nd
    nc.scalar.activation(
        coef[:],
        ndf[:],
        mybir.ActivationFunctionType.Copy,
        bias=GAMMA * LMBDA,
        scale=-GAMMA * LMBDA,
    )

    wb = w_t[:].unsqueeze(1).broadcast_to([P, tc_sz, D])
    for j in range(nchunk):
        ot = opool.tile([P, tc_sz * D], F32)
        nt = npool.tile([P, tc_sz * D], F32)
        fs = slice(j * tc_sz * D, (j + 1) * tc_sz * D)
        nc.sync.dma_start(ot[:], obs_d.ap()[:, fs])
        eng2.dma_start(nt[:], nobs_d.ap()[:, fs])
        if nocompute:
            continue
        o3 = ot[:].rearrange("p (t d) -> p t d", d=D)
        n3 = nt[:].rearrange("p (t d) -> p t d", d=D)
        cs = slice(j * tc_sz, (j + 1) * tc_sz)
        nc.vector.tensor_tensor(out=o3, in0=o3, in1=wb, op=mult)
        nc.vector.tensor_reduce(
            out=v_raw[:, cs], in_=o3, axis=mybir.AxisListType.X, op=add
        )
        nc.gpsimd.tensor_tensor(out=n3, in0=n3, in1=wb, op=mult)
        nc.vector.tensor_reduce(
            out=nv_raw[:, cs], in_=n3, axis=mybir.AxisListType.X, op=add
        )
    if nocompute:
        # still write outputs so the IO footprint matches (garbage values;
        # rw_t is used because it is actually written by a DMA above)
        oeng.dma_start(adv_d.ap(), rw_t[:])
        oeng.dma_start(tgt_d.ap(), rw_t[:])
        return

    # epilogue: td0 = reward + gamma*nd*(nv_raw+b) - (v_raw+b)
    nvb = ppool.tile([P, tp], F32)
    nc.vector.tensor_scalar_add(nvb[:], nv_raw[:], b_t[:, 0:1])
    vb = ppool.tile([P, tp], F32)  # value = v_raw + b
    nc.vector.tensor_scalar_add(vb[:], v_raw[:], b_t[:, 0:1])
    q = ppool.tile([P, tp], F32)
    nc.vector.tensor_tensor(out=q[:], in0=g[:], in1=nvb[:], op=mult)
    s = ppool.tile([P, tp], F32)
    nc.vector.tensor_tensor(out=s[:], in0=rw_t[:], in1=vb[:], op=sub)
    td0 = ppool.tile([P, tp], F32)
    nc.vector.tensor_tensor(out=td0[:], in0=q[:], in1=s[:], op=add)

    # Backward scan: second half (partitions 64..127, later timesteps)
    # first; its t'=0 element is A at the first half's boundary.
    adv = ppool.tile([P, tp], F32)
    hi = slice(BL, 2 * BL)
    lo = slice(0, BL)
    nc.vector.tensor_tensor_scan(
        out=adv[hi, ::-1],
        data0=coef[hi, ::-1],
        data1=td0[hi, ::-1],
        initial=0.0,
        op0=mult,
        op1=add,
    )
    oeng.dma_start(bnd[:], adv[hi, 0:1])
    nc.vector.tensor_tensor_scan(
        out=adv[lo, ::-1],
        data0=coef[lo, ::-1],
        data1=td0[lo, ::-1],
        initial=bnd[:, 0:1],
        op0=mult,
        op1=add,
    )

    tgt = ppool.tile([P, tp], F32)
    nc.vector.tensor_tensor(out=tgt[:], in0=adv[:], in1=vb[:], op=add)

    oeng.dma_start(adv_d.ap(), adv[:])
    oeng.dma_start(tgt_d.ap(), tgt[:])


def build_program(
    t_total=T, nchunk=16, repeat=1, dual_dma=False, nocompute=False, bufs=3,
    bench_internal=False, out_scalar=True, dbl=2, obufs=4,
):
    """Build the per-core Bass program (all 8 cores run it SPMD on their own
    shard). DRAM tensor layouts are (half, batch)-major as produced by
    shard_inputs. repeat>1 re-runs the whole pipeline inside one NEFF
    (test.py uses the delta vs repeat=1 to measure per-iteration HW time).
    bench_internal makes obs/next_obs Internal DRAM (not shipped per call;
    garbage values) so benchmark calls are cheap — timing-only builds."""
    tp = t_total // H  # timesteps per partition
    tc_sz = tp // nchunk  # timesteps per streamed chunk
    assert tp % nchunk == 0

    nc = bacc.Bacc(
        "TRN2", target_bir_lowering=False, debug=False, enable_asserts=False
    )

    big_kind = "Internal" if bench_internal else "ExternalInput"
    obs_d = nc.dram_tensor("obs", [P, tp * D], F32, kind=big_kind)
    nobs_d = nc.dram_tensor("nobs", [P, tp * D], F32, kind=big_kind)
    rw_d = nc.dram_tensor("rw", [P, tp], F32, kind="ExternalInput")
    dn_d = nc.dram_tensor("dn", [P, tp], U8, kind="ExternalInput")
    w_d = nc.dram_tensor("w", [D], F32, kind="ExternalInput")
    b_d = nc.dram_tensor("b", [1], F32, kind="ExternalInput")
    adv_d = nc.dram_tensor("adv", [P, tp], F32, kind="ExternalOutput")
    tgt_d = nc.dram_tensor("tgt", [P, tp], F32, kind="ExternalOutput")

    with tile.TileContext(nc) as tc, ExitStack() as ctx:
        cpool = ctx.enter_context(tc.tile_pool(name="const", bufs=1))
        opool = ctx.enter_context(tc.tile_pool(name="obs", bufs=obufs or bufs))
        npool = ctx.enter_context(tc.tile_pool(name="nobs", bufs=bufs))
        ppool = ctx.enter_context(tc.tile_pool(name="pers", bufs=1))
        dpool = ctx.enter_context(tc.tile_pool(name="dbl", bufs=dbl))

        # Value-head weights replicated to every partition.
        w_t = cpool.tile([P, D], F32)
        nc.sync.dma_start(w_t[:], w_d.ap().unsqueeze(0).broadcast_to([P, D]))
        b_t = cpool.tile([P, 1], F32)
        nc.sync.dma_start(b_t[:], b_d.ap().unsqueeze(0).broadcast_to([P, 1]))

        bnd = cpool.tile([BL, 1], F32)

        for _rep in range(repeat):
            _build_iter(
                nc, opool, npool, ppool, dpool, w_t, b_t, bnd,
                obs_d, nobs_d, rw_d, dn_d, adv_d, tgt_d, tp, tc_sz, nchunk,
                dual_dma=dual_dma, nocompute=nocompute, out_scalar=out_scalar,
            )

    # Runs the bacc pipeline (register allocation etc.) — required before
    # serializing for the walrus compiler.
    nc.finalize()
    return nc


_NC_CACHE = None


def _get_nc():
    global _NC_CACHE
    if _NC_CACHE is None:
        _NC_CACHE = build_program()
    return _NC_CACHE


def _hmajor(x, tp_cols):
    """[BL, H*tp_cols] row-major -> [H*BL, tp_cols] with row p = h*BL + b."""
    return np.ascontiguousarray(
        x.reshape(BL, H, tp_cols).transpose(1, 0, 2).reshape(H * BL, tp_cols)
    )


def _unhmajor(y):
    """Inverse of _hmajor for outputs: [H*BL, tp] -> [BL, H*tp]."""
    tp = y.shape[1]
    return y.reshape(H, BL, tp).transpose(1, 0, 2).reshape(BL, H * tp)


def shard_inputs(obs, next_obs, reward, done, W, b):
    """Split full inputs into the 8 per-core input maps ((h,b)-major)."""
    obs = np.asarray(obs, dtype=np.float32).reshape(B, T * D)
    nobs = np.asarray(next_obs, dtype=np.float32).reshape(B, T * D)
    rw = np.asarray(reward, dtype=np.float32).reshape(B, T)
    dn = np.asarray(done).astype(np.uint8, copy=False).reshape(B, T)
    w_np = np.ascontiguousarray(np.asarray(W, dtype=np.float32)).reshape(D)
    b_np = np.ascontiguousarray(np.asarray(b, dtype=np.float32)).reshape(1)

    tpd = (T // H) * D
    tp = T // H
    in_maps = []
    for i in range(NCORES):
        sl = slice(i * BL, (i + 1) * BL)
        in_maps.append(
            {
                "obs": _hmajor(obs[sl], tpd),
                "nobs": _hmajor(nobs[sl], tpd),
                "rw": _hmajor(rw[sl], tp),
                "dn": _hmajor(dn[sl], tp),
                "w": w_np,
                "b": b_np,
            }
        )
    return in_maps


def gather_outputs(results):
    advantage = np.concatenate(
        [_unhmajor(r["adv"]) for r in results], axis=0
    ).reshape(B, T, 1)
    value_target = np.concatenate(
        [_unhmajor(r["tgt"]) for r in results], axis=0
    ).reshape(B, T, 1)
    return advantage, value_target


def kernel(obs, next_obs, reward, done, W, b):
    global LAST_RESULTS
    nc = _get_nc()
    in_maps = shard_inputs(obs, next_obs, reward, done, W, b)
    res = run_bass_kernel_spmd(nc, in_maps, core_ids=list(range(NCORES)))
    LAST_RESULTS = res
    return gather_outputs(res.results)



# revision 10
# speedup vs baseline: 1.5800x; 1.5800x over previous
"""GAE (generalized advantage estimation) Trainium2 kernel — PE-matmul edition.

Problem: nn_CustomGAE — B=512, T=2048, D=64.
  value = obs @ W + b ; next_value = next_obs @ W + b
  td0 = reward + gamma*nd*next_value - value ; coef = gamma*lambda*nd
  A_t = td0_t + coef_t * A_{t+1}  (reverse scan over T, independent per traj)
  returns (advantage, value_target = advantage + value)

Sharding: pure data parallel over B across 8 cores (64 trajectories/core).

Architecture (vs a DVE mult+reduce formulation, which is DVE-bound): the host
pre-transposes obs/next_obs to D-on-partitions layout obsT[d, (h, b, t')] so
the value-head matvec runs on the otherwise-idle PE (tensor) engine as
float32r matmuls: SBUF chunk tiles hold obsT on partitions 0..63 and nobsT on
partitions 64..127, the stationary is a [128, 2] masked-W pair, and each
matmul emits out[2, 512] = (value, next_value) for 512 (traj-half, t')
columns into one PSUM bank.  The Activation engine evacuates PSUM -> SBUF
staging, and SBUF->SBUF DMAs issued from the (idle) GPSIMD software DGE
scatter each staged row into the scan layout v_raw/nv_raw [128 traj-halves,
1024 t'].  DVE is left with only the O(B*T) epilogue, so the kernel is
DMA/HBM-bound.

Ring discipline: ALL input streaming runs on the SP (sync) HWDGE ring with
waits only on pool-slot availability, so it never stalls behind compute.
Scatter/boundary/output DMAs ride the Pool SWDGE queue so their waits park
the idle Pool engine, not an input ring or the Act ring issuing evacuations.

Tail minimization: the two reverse scans (trajectory halves on partition
blocks) both run seeded with 0; the cross-half boundary term is applied
afterwards as adv_lo += suffix_cumprod(coef_lo) * adv_hi[0] (one fused
scalar_tensor_tensor), with the suffix cumprod hoisted into the stream, so
the boundary DMA is off the critical path.  The bias b is likewise folded
into rwb = reward + b*(gamma*nd - 1) during the stream, leaving a 3-op td0
chain + 2 scans + fix + fused target in the post-stream tail.
"""

import sys

sys.path.insert(0, "/opt/trn_rl_repo")

from contextlib import ExitStack

import ml_dtypes
import numpy as np

import concourse.bacc as bacc
import concourse.mybir as mybir
import concourse.tile as tile
from concourse.bass_utils import run_bass_kernel_spmd

GAMMA = 0.99
LMBDA = 0.95

B, T, D = 512, 2048, 64
NCORES = 8
BL = B // NCORES  # 64 trajectories per core
H = 2  # trajectory halves stacked on partitions -> 128 partitions
P = H * BL  # 128
F32 = mybir.dt.float32
BF16 = mybir.dt.bfloat16
U8 = mybir.dt.uint8

MM_N = 512  # moving columns per matmul = one PSUM bank of f32
GRP_MM = 4  # matmuls per PSUM tile (4 banks); 2 tiles ping-pong = 8 banks
STAGE_GRPS = 2  # PSUM groups evacuated into one staging tile / scatter pair

# Results of the last hardware run, for test harnesses.
LAST_RESULTS = None


def _build_iter(
    nc, kpool, ppool, dpool, spool, qpool, consts,
    obs_d, nobs_d, rw_d, dn_d, adv_d, tgt_d, tp, cc, nchunk,
    nocompute=False,
):
    """One full pass: load inputs, PE matvec, scan, write outputs.

    Columns stream hi-half (h=1, partitions 64..127) first, so the hi-half
    td0 chain, reverse scan, boundary DMA, and hi outputs all run mid-stream;
    only the lo-half chain remains in the post-stream tail."""
    w2_t, b_t, nb_t, zeros_lo, bnd = consts
    mult = mybir.AluOpType.mult
    add = mybir.AluOpType.add
    copyf = mybir.ActivationFunctionType.Copy
    identf = mybir.ActivationFunctionType.Identity
    grp_cols = GRP_MM * MM_N  # 2048
    ngrp_c = cc // grp_cols  # psum groups per chunk
    sg_cols = STAGE_GRPS * grp_cols
    hi = slice(BL, 2 * BL)
    lo = slice(0, BL)

    rw_t = dpool.tile([P, tp], F32)  # becomes rwb = rw + b*(g-1) in place
    dn_t = dpool.tile([P, tp], U8)
    v_raw = dpool.tile([P, tp], F32)
    nv_raw = dpool.tile([P, tp], F32)
    ndf = dpool.tile([P, tp], F32)  # u8 done -> f32; later reused as gb
    g = dpool.tile([P, tp], F32)
    coef = dpool.tile([P, tp], F32)
    pfx = dpool.tile([BL, tp], F32)  # suffix cumprod of coef (low half)
    q = ppool.tile([P, tp], F32)
    s = ppool.tile([P, tp], F32)
    td0 = ppool.tile([P, tp], F32)
    adv = ppool.tile([P, tp], F32)
    tgt = ppool.tile([P, tp], F32)

    def td0_half(h):
        # td0 = g*nv + (rwb - v) on one partition half
        nc.vector.tensor_tensor(out=q[h, :], in0=g[h, :], in1=nv_raw[h, :], op=mult)
        nc.vector.scalar_tensor_tensor(
            out=s[h, :], in0=v_raw[h, :], scalar=-1.0, in1=rw_t[h, :],
            op0=mult, op1=add,
        )
        nc.vector.tensor_tensor(out=td0[h, :], in0=q[h, :], in1=s[h, :], op=add)

    def scan_half(h):
        nc.vector.tensor_tensor_scan(
            out=adv[h, ::-1],
            data0=coef[h, ::-1],
            data1=td0[h, ::-1],
            initial=0.0,
            op0=mult,
            op1=add,
        )

    st = None
    for j in range(nchunk):
        ct = kpool.tile([P, cc], BF16)
        fs = slice(j * cc, (j + 1) * cc)
        nc.sync.dma_start(ct[0:BL, :], obs_d.ap()[:, fs])
        nc.sync.dma_start(ct[BL:P, :], nobs_d.ap()[:, fs])
        if j == min(1, nchunk - 1):
            # Small inputs ride the SP ring behind the first chunks; the
            # nd-derived factors then overlap the rest of the stream.
            nc.sync.dma_start(rw_t[:], rw_d.ap())
            nc.sync.dma_start(dn_t[:], dn_d.ap())
            if not nocompute:
                nc.vector.tensor_copy(ndf[:], dn_t[:])
                nc.scalar.activation(g[:], ndf[:], copyf, bias=GAMMA, scale=-GAMMA)
                nc.scalar.activation(
                    coef[:], ndf[:], copyf,
                    bias=GAMMA * LMBDA, scale=-GAMMA * LMBDA,
                )
                # gb = b*g - b (reuses ndf storage); rwb = rw + gb in place.
                nc.scalar.activation(
                    ndf[:], g[:], identf, bias=nb_t[:, 0:1], scale=b_t[:, 0:1]
                )
                nc.vector.tensor_tensor(out=rw_t[:], in0=rw_t[:], in1=ndf[:], op=add)
                # pfx[t'] = prod_{s>=t'} coef_lo[s]
                nc.vector.tensor_tensor_scan(
                    out=pfx[:, ::-1],
                    data0=coef[lo, ::-1],
                    data1=zeros_lo[:],
                    initial=1.0,
                    op0=mult,
                    op1=add,
                )
        if not nocompute:
            for gi in range(ngrp_c):
                gg = j * ngrp_c + gi  # global psum-group index
                si = gg % STAGE_GRPS
                if si == 0:
                    st = spool.tile([2, sg_cols], F32)
                pt = qpool.tile([2, grp_cols], F32)
                for k in range(GRP_MM):
                    cs = slice(
                        (gi * GRP_MM + k) * MM_N, (gi * GRP_MM + k + 1) * MM_N
                    )
                    nc.tensor.matmul(
                        pt[:, k * MM_N : (k + 1) * MM_N],
                        lhsT=w2_t[:],
                        rhs=ct[:, cs],
                        start=True,
                        stop=True,
                    )
                if gg % 5 < 3:
                    nc.scalar.activation(
                        st[:, si * grp_cols : (si + 1) * grp_cols], pt[:], copyf
                    )
                else:
                    nc.vector.tensor_copy(
                        st[:, si * grp_cols : (si + 1) * grp_cols], pt[:]
                    )
                if si == STAGE_GRPS - 1:
                    # Scatter staged rows into scan layout (Pool SWDGE: waits
                    # on the evacuation park the idle Pool queue only).
                    # Streaming order is hi-half first: staged trajectory-half
                    # index thn maps to partition 64+thn (hi) / thn-64 (lo).
                    thn = (gg + 1 - STAGE_GRPS) * (grp_cols // tp)
                    npart = sg_cols // tp
                    dst = BL + thn if thn < BL else thn - BL
                    nc.gpsimd.dma_start(
                        v_raw[dst : dst + npart, :], st[0:1, :]
                    )
                    nc.gpsimd.dma_start(
                        nv_raw[dst : dst + npart, :], st[1:2, :]
                    )
            if j == nchunk // 2 - 1:
                # hi half fully scattered (once those DMAs land): run its
                # td0 chain + reverse scan in-stream on the idle DVE.
                td0_half(hi)
                scan_half(hi)
                nc.vector.scalar_tensor_tensor(
                    out=tgt[hi, :], in0=v_raw[hi, :], scalar=b_t[hi, 0:1],
                    in1=adv[hi, :], op0=add, op1=add,
                )
            if j == min(nchunk // 2 + 1, nchunk - 1):
                # Boundary + hi outputs, emitted a couple chunks later so
                # their waits are already satisfied when Pool reaches them.
                nc.gpsimd.dma_start(bnd[:], adv[hi, 0:1])
                nc.gpsimd.dma_start(adv_d.ap()[hi, :], adv[hi, :])
                nc.gpsimd.dma_start(tgt_d.ap()[hi, :], tgt[hi, :])

    if nocompute:
        # still write outputs so the IO footprint matches (garbage values;
        # rw_t is used because it is actually written by a DMA above)
        nc.gpsimd.dma_start(adv_d.ap(), rw_t[:])
        nc.gpsimd.dma_start(tgt_d.ap(), rw_t[:])
        return

    # Post-stream tail: lo-half td0 chain, zero-seeded scan, boundary patch
    # via the in-stream suffix cumprod, fused target, outputs.
    td0_half(lo)
    scan_half(lo)
    nc.vector.scalar_tensor_tensor(
        out=adv[lo, :], in0=pfx[:], scalar=bnd[:, 0:1], in1=adv[lo, :],
        op0=mult, op1=add,
    )
    nc.gpsimd.dma_start(adv_d.ap()[lo, :], adv[lo, :])
    # value_target = adv + value = (v_raw + b) + adv, fused.
    nc.vector.scalar_tensor_tensor(
        out=tgt[lo, :], in0=v_raw[lo, :], scalar=b_t[lo, 0:1], in1=adv[lo, :],
        op0=add, op1=add,
    )
    nc.gpsimd.dma_start(tgt_d.ap()[lo, :], tgt[lo, :])


def build_program(
    t_total=T, nchunk=None, repeat=1, nocompute=False, bufs=3,
    bench_internal=False, chunk_cols=4096, sbufs=4,
):
    """Build the per-core Bass program (all 8 cores run it SPMD on their own
    shard). obs/nobs DRAM layout is [D, (h, b, t')] (host pre-transposed);
    rw/dn/adv/tgt are [(h, b), t'] as in the baseline. repeat>1 re-runs the
    whole pipeline inside one NEFF (test.py uses the delta vs repeat=1 to
    measure per-iteration HW time). bench_internal makes obs/next_obs
    Internal DRAM (not shipped per call; garbage values) so benchmark calls
    are cheap — timing-only builds."""
    tp = t_total // H  # timesteps per partition
    ncols = BL * t_total  # matmul columns
    assert ncols % chunk_cols == 0
    nchunk = ncols // chunk_cols
    assert chunk_cols % (GRP_MM * MM_N) == 0
    assert (GRP_MM * MM_N) % tp == 0, "groups must cover whole traj-halves"

    nc = bacc.Bacc(
        "TRN2", target_bir_lowering=False, debug=False, enable_asserts=False
    )

    big_kind = "Internal" if bench_internal else "ExternalInput"
    obs_d = nc.dram_tensor("obs", [BL, ncols], BF16, kind=big_kind)
    nobs_d = nc.dram_tensor("nobs", [BL, ncols], BF16, kind=big_kind)
    rw_d = nc.dram_tensor("rw", [P, tp], F32, kind="ExternalInput")
    dn_d = nc.dram_tensor("dn", [P, tp], U8, kind="ExternalInput")
    w2_d = nc.dram_tensor("w2", [P, 2], BF16, kind="ExternalInput")
    b_d = nc.dram_tensor("b", [1], F32, kind="ExternalInput")
    adv_d = nc.dram_tensor("adv", [P, tp], F32, kind="ExternalOutput")
    tgt_d = nc.dram_tensor("tgt", [P, tp], F32, kind="ExternalOutput")

    with tile.TileContext(nc) as tc, ExitStack() as ctx:
        cpool = ctx.enter_context(tc.tile_pool(name="const", bufs=1))
        kpool = ctx.enter_context(tc.tile_pool(name="chunks", bufs=bufs))
        ppool = ctx.enter_context(tc.tile_pool(name="pers", bufs=1))
        dpool = ctx.enter_context(tc.tile_pool(name="dbl", bufs=2))
        spool = ctx.enter_context(tc.tile_pool(name="stage", bufs=sbufs))
        qpool = ctx.enter_context(tc.psum_pool(name="psum", bufs=2))

        # Masked value-head weight pair: col 0 selects obs (parts 0..63),
        # col 1 selects next_obs (parts 64..127).
        w2_t = cpool.tile([P, 2], BF16)
        nc.sync.dma_start(w2_t[:], w2_d.ap())
        b_t = cpool.tile([P, 1], F32)
        nc.sync.dma_start(b_t[:], b_d.ap().unsqueeze(0).broadcast_to([P, 1]))
        nb_t = cpool.tile([P, 1], F32)  # -b, bias operand for gb
        nc.scalar.activation(
            nb_t[:], b_t[:], mybir.ActivationFunctionType.Copy,
            bias=0.0, scale=-1.0,
        )
        zeros_lo = cpool.tile([BL, tp], F32)
        nc.vector.memset(zeros_lo[:], 0.0)
        bnd = cpool.tile([BL, 1], F32)
        consts = (w2_t, b_t, nb_t, zeros_lo, bnd)

        for _rep in range(repeat):
            _build_iter(
                nc, kpool, ppool, dpool, spool, qpool, consts,
                obs_d, nobs_d, rw_d, dn_d, adv_d, tgt_d, tp, chunk_cols,
                nchunk, nocompute=nocompute,
            )

    # Runs the bacc pipeline (register allocation etc.) — required before
    # serializing for the walrus compiler.
    nc.finalize()
    return nc


_NC_CACHE = None


def _get_nc():
    global _NC_CACHE
    if _NC_CACHE is None:
        _NC_CACHE = build_program()
    return _NC_CACHE


def _hmajor(x, tp_cols):
    """[BL, H*tp_cols] row-major -> [H*BL, tp_cols] with row p = h*BL + b."""
    return np.ascontiguousarray(
        x.reshape(BL, H, tp_cols).transpose(1, 0, 2).reshape(H * BL, tp_cols)
    )


def _unhmajor(y):
    """Inverse of _hmajor for outputs: [H*BL, tp] -> [BL, H*tp]."""
    tp = y.shape[1]
    return y.reshape(H, BL, tp).transpose(1, 0, 2).reshape(BL, H * tp)


def _dmajor(x, tp):
    """[BL, T, D] bf16 -> [D, ncols] with col n = (1-h, b, t'): hi first."""
    return np.ascontiguousarray(
        x.reshape(BL, H, tp, D)[:, ::-1].transpose(3, 1, 0, 2).reshape(
            D, BL * H * tp
        )
    )


def shard_inputs(obs, next_obs, reward, done, W, b):
    """Split full inputs into the 8 per-core input maps."""
    obs = np.asarray(obs, dtype=np.float32).reshape(B, T, D)
    nobs = np.asarray(next_obs, dtype=np.float32).reshape(B, T, D)
    obs = obs.astype(ml_dtypes.bfloat16)
    nobs = nobs.astype(ml_dtypes.bfloat16)
    rw = np.asarray(reward, dtype=np.float32).reshape(B, T)
    dn = np.asarray(done).astype(np.uint8, copy=False).reshape(B, T)
    w_np = np.ascontiguousarray(np.asarray(W, dtype=np.float32)).reshape(D)
    b_np = np.ascontiguousarray(np.asarray(b, dtype=np.float32)).reshape(1)

    w2 = np.zeros((P, 2), ml_dtypes.bfloat16)
    w2[0:BL, 0] = w_np
    w2[BL:P, 1] = w_np

    tp = T // H
    in_maps = []
    for i in range(NCORES):
        sl = slice(i * BL, (i + 1) * BL)
        in_maps.append(
            {
                "obs": _dmajor(obs[sl], tp),
                "nobs": _dmajor(nobs[sl], tp),
                "rw": _hmajor(rw[sl], tp),
                "dn": _hmajor(dn[sl], tp),
                "w2": w2,
                "b": b_np,
            }
        )
    return in_maps


def gather_outputs(results):
    advantage = np.concatenate(
        [_unhmajor(r["adv"]) for r in results], axis=0
    ).reshape(B, T, 1)
    value_target = np.concatenate(
        [_unhmajor(r["tgt"]) for r in results], axis=0
    ).reshape(B, T, 1)
    return advantage, value_target


def kernel(obs, next_obs, reward, done, W, b):
    global LAST_RESULTS
    nc = _get_nc()
    in_maps = shard_inputs(obs, next_obs, reward, done, W, b)
    res = run_bass_kernel_spmd(nc, in_maps, core_ids=list(range(NCORES)))
    LAST_RESULTS = res
    return gather_outputs(res.results)


# revision 12
# speedup vs baseline: 1.9253x; 1.2186x over previous
"""GAE (generalized advantage estimation) Trainium2 kernel — PE-matmul edition.

Problem: nn_CustomGAE — B=512, T=2048, D=64.
  value = obs @ W + b ; next_value = next_obs @ W + b
  td0 = reward + gamma*nd*next_value - value ; coef = gamma*lambda*nd
  A_t = td0_t + coef_t * A_{t+1}  (reverse scan over T, independent per traj)
  returns (advantage, value_target = advantage + value)

Sharding: pure data parallel over B across 8 cores (64 trajectories/core).

Architecture (vs a DVE mult+reduce formulation, which is DVE-bound): the host
pre-transposes obs/next_obs to D-on-partitions layout obsT[d, (h, b, t')] so
the value-head matvec runs on the otherwise-idle PE (tensor) engine as
float32r matmuls: SBUF chunk tiles hold obsT on partitions 0..63 and nobsT on
partitions 64..127, the stationary is a [128, 2] masked-W pair, and each
matmul emits out[2, 512] = (value, next_value) for 512 (traj-half, t')
columns into one PSUM bank.  The Activation engine evacuates PSUM -> SBUF
staging, and SBUF->SBUF DMAs issued from the (idle) GPSIMD software DGE
scatter each staged row into the scan layout v_raw/nv_raw [128 traj-halves,
1024 t'].  DVE is left with only the O(B*T) epilogue, so the kernel is
DMA/HBM-bound.

Ring discipline: ALL input streaming runs on the SP (sync) HWDGE ring with
waits only on pool-slot availability, so it never stalls behind compute.
Scatter/boundary/output DMAs ride the Pool SWDGE queue so their waits park
the idle Pool engine, not an input ring or the Act ring issuing evacuations.

Tail minimization: the two reverse scans (trajectory halves on partition
blocks) both run seeded with 0; the cross-half boundary term is applied
afterwards as adv_lo += suffix_cumprod(coef_lo) * adv_hi[0] (one fused
scalar_tensor_tensor), with the suffix cumprod hoisted into the stream, so
the boundary DMA is off the critical path.  The bias b is likewise folded
into rwb = reward + b*(gamma*nd - 1) during the stream, leaving a 3-op td0
chain + 2 scans + fix + fused target in the post-stream tail.
"""

import sys

sys.path.insert(0, "/opt/trn_rl_repo")

from contextlib import ExitStack

import ml_dtypes
import numpy as np

import concourse.bacc as bacc
import concourse.mybir as mybir
import concourse.tile as tile
from concourse.bass_utils import run_bass_kernel_spmd

GAMMA = 0.99
LMBDA = 0.95

B, T, D = 512, 2048, 64
NCORES = 8
BL = B // NCORES  # 64 trajectories per core
H = 2  # trajectory halves stacked on partitions -> 128 partitions
P = H * BL  # 128
F32 = mybir.dt.float32
BF16 = mybir.dt.bfloat16
U8 = mybir.dt.uint8

MM_N = 512  # moving columns per matmul = one PSUM bank of f32
GRP_MM = 4  # matmuls per PSUM tile (4 banks); 2 tiles ping-pong = 8 banks
STAGE_GRPS = 2  # PSUM groups evacuated into one staging tile / scatter pair

# Results of the last hardware run, for test harnesses.
LAST_RESULTS = None


def _build_iter(
    nc, kpool, ppool, dpool, spool, qpool, consts,
    onz_d, rw_d, dn_d, adv_d, tgt_d, tp, cc, nchunk,
    nocompute=False,
):
    """One full pass: load inputs, PE matvec, scan, write outputs.

    Columns stream hi-half (h=1, partitions 64..127) first, so the hi-half
    td0 chain, reverse scan, boundary DMA, and hi outputs all run mid-stream;
    only the lo-half chain remains in the post-stream tail."""
    w2_t, b_t, nb_t, zeros_lo, bnd = consts
    mult = mybir.AluOpType.mult
    add = mybir.AluOpType.add
    copyf = mybir.ActivationFunctionType.Copy
    identf = mybir.ActivationFunctionType.Identity
    grp_cols = GRP_MM * MM_N  # 2048
    ngrp_c = cc // grp_cols  # psum groups per chunk
    sg_cols = STAGE_GRPS * grp_cols
    hi = slice(BL, 2 * BL)
    lo = slice(0, BL)

    rw_t = dpool.tile([P, tp], F32)  # becomes rwb = rw + b*(g-1) in place
    dn_t = dpool.tile([P, tp], U8)
    v_raw = dpool.tile([P, tp], F32)
    nv_raw = dpool.tile([P, tp], F32)
    ndf = dpool.tile([P, tp], F32)  # u8 done -> f32; later reused as gb
    g = dpool.tile([P, tp], F32)
    coef = dpool.tile([P, tp], F32)
    pfx = dpool.tile([BL, tp], F32)  # suffix cumprod of coef (low half)
    q = ppool.tile([P, tp], F32)
    s = ppool.tile([P, tp], F32)
    td0 = ppool.tile([P, tp], F32)
    adv = ppool.tile([P, tp], F32)
    tgt = ppool.tile([P, tp], F32)

    def td0_half(h):
        # td0 = g*nv + (rwb - v) on one partition half
        nc.vector.tensor_tensor(out=q[h, :], in0=g[h, :], in1=nv_raw[h, :], op=mult)
        nc.vector.scalar_tensor_tensor(
            out=s[h, :], in0=v_raw[h, :], scalar=-1.0, in1=rw_t[h, :],
            op0=mult, op1=add,
        )
        nc.vector.tensor_tensor(out=td0[h, :], in0=q[h, :], in1=s[h, :], op=add)

    def scan_half(h):
        nc.vector.tensor_tensor_scan(
            out=adv[h, ::-1],
            data0=coef[h, ::-1],
            data1=td0[h, ::-1],
            initial=0.0,
            op0=mult,
            op1=add,
        )

    st = None
    for j in range(nchunk):
        ct = kpool.tile([P, cc], BF16)
        fs = slice(j * cc, (j + 1) * cc)
        nc.sync.dma_start(ct[:], onz_d.ap()[:, fs])
        if j == min(1, max(0, nchunk // 2 - 1)):
            # Small inputs ride the SP ring behind the first chunks; the
            # nd-derived factors then overlap the rest of the stream.
            nc.sync.dma_start(rw_t[:], rw_d.ap())
            nc.sync.dma_start(dn_t[:], dn_d.ap())
            if not nocompute:
                nc.vector.tensor_copy(ndf[:], dn_t[:])
                nc.scalar.activation(g[:], ndf[:], copyf, bias=GAMMA, scale=-GAMMA)
                nc.scalar.activation(
                    coef[:], ndf[:], copyf,
                    bias=GAMMA * LMBDA, scale=-GAMMA * LMBDA,
                )
                # gb = b*g - b (reuses ndf storage); rwb = rw + gb in place.
                nc.scalar.activation(
                    ndf[:], g[:], identf, bias=nb_t[:, 0:1], scale=b_t[:, 0:1]
                )
                nc.vector.tensor_tensor(out=rw_t[:], in0=rw_t[:], in1=ndf[:], op=add)
                # pfx[t'] = prod_{s>=t'} coef_lo[s]
                nc.vector.tensor_tensor_scan(
                    out=pfx[:, ::-1],
                    data0=coef[lo, ::-1],
                    data1=zeros_lo[:],
                    initial=1.0,
                    op0=mult,
                    op1=add,
                )
        if not nocompute:
            for gi in range(ngrp_c):
                gg = j * ngrp_c + gi  # global psum-group index
                si = gg % STAGE_GRPS
                if si == 0:
                    st = spool.tile([2, sg_cols], F32)
                pt = qpool.tile([2, grp_cols], F32)
                for k in range(GRP_MM):
                    cs = slice(
                        (gi * GRP_MM + k) * MM_N, (gi * GRP_MM + k + 1) * MM_N
                    )
                    nc.tensor.matmul(
                        pt[:, k * MM_N : (k + 1) * MM_N],
                        lhsT=w2_t[:],
                        rhs=ct[:, cs],
                        start=True,
                        stop=True,
                    )
                if gg % 5 < 3:
                    nc.scalar.activation(
                        st[:, si * grp_cols : (si + 1) * grp_cols], pt[:], copyf
                    )
                else:
                    nc.vector.tensor_copy(
                        st[:, si * grp_cols : (si + 1) * grp_cols], pt[:]
                    )
                if si == STAGE_GRPS - 1:
                    # Scatter staged rows into scan layout (Pool SWDGE: waits
                    # on the evacuation park the idle Pool queue only).
                    # Streaming order is hi-half first: staged trajectory-half
                    # index thn maps to partition 64+thn (hi) / thn-64 (lo).
                    thn = (gg + 1 - STAGE_GRPS) * (grp_cols // tp)
                    npart = sg_cols // tp
                    dst = BL + thn if thn < BL else thn - BL
                    nc.gpsimd.dma_start(
                        v_raw[dst : dst + npart, :], st[0:1, :]
                    )
                    nc.gpsimd.dma_start(
                        nv_raw[dst : dst + npart, :], st[1:2, :]
                    )
            if j == nchunk // 2 - 1:
                # hi half fully scattered (once those DMAs land): run its
                # td0 chain + reverse scan in-stream on the idle DVE.
                td0_half(hi)
                scan_half(hi)
                nc.vector.scalar_tensor_tensor(
                    out=tgt[hi, :], in0=v_raw[hi, :], scalar=b_t[hi, 0:1],
                    in1=adv[hi, :], op0=add, op1=add,
                )
            if j == min(nchunk // 2 + 1, nchunk - 1):
                # Boundary + hi outputs, emitted a couple chunks later so
                # their waits are already satisfied when Pool reaches them.
                nc.gpsimd.dma_start(bnd[:], adv[hi, 0:1])
                nc.gpsimd.dma_start(adv_d.ap()[hi, :], adv[hi, :])
                nc.gpsimd.dma_start(tgt_d.ap()[hi, :], tgt[hi, :])

    if nocompute:
        # still write outputs so the IO footprint matches (garbage values;
        # rw_t is used because it is actually written by a DMA above)
        nc.gpsimd.dma_start(adv_d.ap(), rw_t[:])
        nc.gpsimd.dma_start(tgt_d.ap(), rw_t[:])
        return

    # Post-stream tail: lo-half td0 chain, zero-seeded scan, boundary patch
    # via the in-stream suffix cumprod, fused target, outputs.
    td0_half(lo)
    scan_half(lo)
    nc.vector.scalar_tensor_tensor(
        out=adv[lo, :], in0=pfx[:], scalar=bnd[:, 0:1], in1=adv[lo, :],
        op0=mult, op1=add,
    )
    nc.gpsimd.dma_start(adv_d.ap()[lo, :], adv[lo, :])
    # value_target = adv + value = (v_raw + b) + adv, fused.
    nc.vector.scalar_tensor_tensor(
        out=tgt[lo, :], in0=v_raw[lo, :], scalar=b_t[lo, 0:1], in1=adv[lo, :],
        op0=add, op1=add,
    )
    nc.gpsimd.dma_start(tgt_d.ap()[lo, :], tgt[lo, :])


def build_program(
    t_total=T, nchunk=None, repeat=1, nocompute=False, bufs=3,
    bench_internal=False, chunk_cols=8192, sbufs=4,
):
    """Build the per-core Bass program (all 8 cores run it SPMD on their own
    shard). obs/nobs DRAM layout is [D, (h, b, t')] (host pre-transposed);
    rw/dn/adv/tgt are [(h, b), t'] as in the baseline. repeat>1 re-runs the
    whole pipeline inside one NEFF (test.py uses the delta vs repeat=1 to
    measure per-iteration HW time). bench_internal makes obs/next_obs
    Internal DRAM (not shipped per call; garbage values) so benchmark calls
    are cheap — timing-only builds."""
    tp = t_total // H  # timesteps per partition
    ncols = BL * t_total  # matmul columns
    assert ncols % chunk_cols == 0
    nchunk = ncols // chunk_cols
    assert chunk_cols % (GRP_MM * MM_N) == 0
    assert (GRP_MM * MM_N) % tp == 0, "groups must cover whole traj-halves"

    nc = bacc.Bacc(
        "TRN2", target_bir_lowering=False, debug=False, enable_asserts=False
    )

    big_kind = "Internal" if bench_internal else "ExternalInput"
    onz_d = nc.dram_tensor("onz", [P, ncols], BF16, kind=big_kind)
    rw_d = nc.dram_tensor("rw", [P, tp], F32, kind="ExternalInput")
    dn_d = nc.dram_tensor("dn", [P, tp], U8, kind="ExternalInput")
    w2_d = nc.dram_tensor("w2", [P, 2], BF16, kind="ExternalInput")
    b_d = nc.dram_tensor("b", [1], F32, kind="ExternalInput")
    adv_d = nc.dram_tensor("adv", [P, tp], F32, kind="ExternalOutput")
    tgt_d = nc.dram_tensor("tgt", [P, tp], F32, kind="ExternalOutput")

    with tile.TileContext(nc) as tc, ExitStack() as ctx:
        cpool = ctx.enter_context(tc.tile_pool(name="const", bufs=1))
        kpool = ctx.enter_context(tc.tile_pool(name="chunks", bufs=bufs))
        ppool = ctx.enter_context(tc.tile_pool(name="pers", bufs=1))
        dpool = ctx.enter_context(tc.tile_pool(name="dbl", bufs=2))
        spool = ctx.enter_context(tc.tile_pool(name="stage", bufs=sbufs))
        qpool = ctx.enter_context(tc.psum_pool(name="psum", bufs=2))

        # Masked value-head weight pair: col 0 selects obs (parts 0..63),
        # col 1 selects next_obs (parts 64..127).
        w2_t = cpool.tile([P, 2], BF16)
        nc.sync.dma_start(w2_t[:], w2_d.ap())
        b_t = cpool.tile([P, 1], F32)
        nc.sync.dma_start(b_t[:], b_d.ap().unsqueeze(0).broadcast_to([P, 1]))
        nb_t = cpool.tile([P, 1], F32)  # -b, bias operand for gb
        nc.scalar.activation(
            nb_t[:], b_t[:], mybir.ActivationFunctionType.Copy,
            bias=0.0, scale=-1.0,
        )
        zeros_lo = cpool.tile([BL, tp], F32)
        nc.vector.memset(zeros_lo[:], 0.0)
        bnd = cpool.tile([BL, 1], F32)
        consts = (w2_t, b_t, nb_t, zeros_lo, bnd)

        for _rep in range(repeat):
            _build_iter(
                nc, kpool, ppool, dpool, spool, qpool, consts,
                onz_d, rw_d, dn_d, adv_d, tgt_d, tp, chunk_cols,
                nchunk, nocompute=nocompute,
            )

    # Runs the bacc pipeline (register allocation etc.) — required before
    # serializing for the walrus compiler.
    nc.finalize()
    return nc


_NC_CACHE = None


def _get_nc():
    global _NC_CACHE
    if _NC_CACHE is None:
        _NC_CACHE = build_program()
    return _NC_CACHE


def _hmajor(x, tp_cols):
    """[BL, H*tp_cols] row-major -> [H*BL, tp_cols] with row p = h*BL + b."""
    return np.ascontiguousarray(
        x.reshape(BL, H, tp_cols).transpose(1, 0, 2).reshape(H * BL, tp_cols)
    )


def _unhmajor(y):
    """Inverse of _hmajor for outputs: [H*BL, tp] -> [BL, H*tp]."""
    tp = y.shape[1]
    return y.reshape(H, BL, tp).transpose(1, 0, 2).reshape(BL, H * tp)


def _dmajor(x, tp):
    """[BL, T, D] bf16 -> [D, ncols] with col n = (1-h, b, t'): hi first."""
    return np.ascontiguousarray(
        x.reshape(BL, H, tp, D)[:, ::-1].transpose(3, 1, 0, 2).reshape(
            D, BL * H * tp
        )
    )


def shard_inputs(obs, next_obs, reward, done, W, b):
    """Split full inputs into the 8 per-core input maps."""
    obs = np.asarray(obs, dtype=np.float32).reshape(B, T, D)
    nobs = np.asarray(next_obs, dtype=np.float32).reshape(B, T, D)
    obs = obs.astype(ml_dtypes.bfloat16)
    nobs = nobs.astype(ml_dtypes.bfloat16)
    rw = np.asarray(reward, dtype=np.float32).reshape(B, T)
    dn = np.asarray(done).astype(np.uint8, copy=False).reshape(B, T)
    w_np = np.ascontiguousarray(np.asarray(W, dtype=np.float32)).reshape(D)
    b_np = np.ascontiguousarray(np.asarray(b, dtype=np.float32)).reshape(1)

    w2 = np.zeros((P, 2), ml_dtypes.bfloat16)
    w2[0:BL, 0] = w_np
    w2[BL:P, 1] = w_np

    tp = T // H
    in_maps = []
    for i in range(NCORES):
        sl = slice(i * BL, (i + 1) * BL)
        in_maps.append(
            {
                "onz": np.concatenate(
                    [_dmajor(obs[sl], tp), _dmajor(nobs[sl], tp)], axis=0
                ),
                "rw": _hmajor(rw[sl], tp),
                "dn": _hmajor(dn[sl], tp),
                "w2": w2,
                "b": b_np,
            }
        )
    return in_maps


def gather_outputs(results):
    advantage = np.concatenate(
        [_unhmajor(r["adv"]) for r in results], axis=0
    ).reshape(B, T, 1)
    value_target = np.concatenate(
        [_unhmajor(r["tgt"]) for r in results], axis=0
    ).reshape(B, T, 1)
    return advantage, value_target


def kernel(obs, next_obs, reward, done, W, b):
    global LAST_RESULTS
    nc = _get_nc()
    in_maps = shard_inputs(obs, next_obs, reward, done, W, b)
    res = run_bass_kernel_spmd(nc, in_maps, core_ids=list(range(NCORES)))
    LAST_RESULTS = res
    return gather_outputs(res.results)


# revision 15
# speedup vs baseline: 2.0758x; 1.0782x over previous
"""GAE (generalized advantage estimation) Trainium2 kernel — PE-matmul edition.

Problem: nn_CustomGAE — B=512, T=2048, D=64.
  value = obs @ W + b ; next_value = next_obs @ W + b
  td0 = reward + gamma*nd*next_value - value ; coef = gamma*lambda*nd
  A_t = td0_t + coef_t * A_{t+1}  (reverse scan over T, independent per traj)
  returns (advantage, value_target = advantage + value)

Sharding: pure data parallel over B across 8 cores (64 trajectories/core).

Architecture (vs a DVE mult+reduce formulation, which is DVE-bound): the host
pre-transposes obs/next_obs to D-on-partitions layout obsT[d, (h, b, t')] so
the value-head matvec runs on the otherwise-idle PE (tensor) engine as
float32r matmuls: SBUF chunk tiles hold obsT on partitions 0..63 and nobsT on
partitions 64..127, the stationary is a [128, 2] masked-W pair, and each
matmul emits out[2, 512] = (value, next_value) for 512 (traj-half, t')
columns into one PSUM bank.  The Activation engine evacuates PSUM -> SBUF
staging, and SBUF->SBUF DMAs issued from the (idle) GPSIMD software DGE
scatter each staged row into the scan layout v_raw/nv_raw [128 traj-halves,
1024 t'].  DVE is left with only the O(B*T) epilogue, so the kernel is
DMA/HBM-bound.

Ring discipline: ALL input streaming runs on the SP (sync) HWDGE ring with
waits only on pool-slot availability, so it never stalls behind compute.
Scatter/boundary/output DMAs ride the Pool SWDGE queue so their waits park
the idle Pool engine, not an input ring or the Act ring issuing evacuations.

Tail minimization: the two reverse scans (trajectory halves on partition
blocks) both run seeded with 0; the cross-half boundary term is applied
afterwards as adv_lo += suffix_cumprod(coef_lo) * adv_hi[0] (one fused
scalar_tensor_tensor), with the suffix cumprod hoisted into the stream, so
the boundary DMA is off the critical path.  The bias b is likewise folded
into rwb = reward + b*(gamma*nd - 1) during the stream, leaving a 3-op td0
chain + 2 scans + fix + fused target in the post-stream tail.
"""

import sys

sys.path.insert(0, "/opt/trn_rl_repo")

from contextlib import ExitStack

import ml_dtypes
import numpy as np

import concourse.bacc as bacc
import concourse.mybir as mybir
import concourse.tile as tile
from concourse.bass_utils import run_bass_kernel_spmd

GAMMA = 0.99
LMBDA = 0.95

B, T, D = 512, 2048, 64
NCORES = 8
BL = B // NCORES  # 64 trajectories per core
H = 2  # trajectory halves stacked on partitions -> 128 partitions
P = H * BL  # 128
F32 = mybir.dt.float32
BF16 = mybir.dt.bfloat16
U8 = mybir.dt.uint8

MM_N = 512  # moving columns per matmul = one PSUM bank of f32
GRP_MM = 4  # matmuls per PSUM tile (4 banks); 2 tiles ping-pong = 8 banks
STAGE_GRPS = 2  # PSUM groups evacuated into one staging tile / scatter pair

# Results of the last hardware run, for test harnesses.
LAST_RESULTS = None


def _build_iter(
    nc, kpool, ppool, dpool, spool, qpool, consts,
    onz_d, rw_d, dn_d, adv_d, tgt_d, tp, cc, nchunk,
    nocompute=False, grp_mm=GRP_MM, stage_grps=STAGE_GRPS,
):
    """One full pass: load inputs, PE matvec, scan, write outputs.

    Columns stream hi-half (h=1, partitions 64..127) first, so the hi-half
    td0 chain, reverse scan, boundary DMA, and hi outputs all run mid-stream;
    only the lo-half chain remains in the post-stream tail."""
    w2_t, b_t, nb_t, zeros_lo, bnd = consts
    mult = mybir.AluOpType.mult
    add = mybir.AluOpType.add
    copyf = mybir.ActivationFunctionType.Copy
    identf = mybir.ActivationFunctionType.Identity
    grp_cols = grp_mm * MM_N
    ngrp_c = cc // grp_cols  # psum groups per chunk
    sg_cols = stage_grps * grp_cols
    hi = slice(BL, 2 * BL)
    lo = slice(0, BL)

    rw_t = dpool.tile([P, tp], F32)  # becomes rwb = rw + b*(g-1) in place
    dn_t = dpool.tile([P, tp], U8)
    v_raw = dpool.tile([P, tp], F32)
    nv_raw = dpool.tile([P, tp], F32)
    ndf = dpool.tile([P, tp], F32)  # u8 done -> f32; later reused as gb
    g = dpool.tile([P, tp], F32)
    coef = dpool.tile([P, tp], F32)
    pfx = dpool.tile([BL, tp], F32)  # suffix cumprod of coef (low half)
    q = ppool.tile([P, tp], F32)
    s = ppool.tile([P, tp], F32)
    td0 = ppool.tile([P, tp], F32)
    adv = ppool.tile([P, tp], F32)
    tgt = ppool.tile([P, tp], F32)

    def td0_half(h):
        # td0 = g*nv + (rwb - v) on one partition half
        nc.vector.tensor_tensor(out=q[h, :], in0=g[h, :], in1=nv_raw[h, :], op=mult)
        nc.vector.scalar_tensor_tensor(
            out=s[h, :], in0=v_raw[h, :], scalar=-1.0, in1=rw_t[h, :],
            op0=mult, op1=add,
        )
        nc.vector.tensor_tensor(out=td0[h, :], in0=q[h, :], in1=s[h, :], op=add)

    def scan_half(h):
        nc.vector.tensor_tensor_scan(
            out=adv[h, ::-1],
            data0=coef[h, ::-1],
            data1=td0[h, ::-1],
            initial=0.0,
            op0=mult,
            op1=add,
        )

    st = None
    for j in range(nchunk):
        ct = kpool.tile([P, cc], BF16)
        fs = slice(j * cc, (j + 1) * cc)
        nc.sync.dma_start(ct[:], onz_d.ap()[:, fs])
        if j == min(1, max(0, nchunk // 2 - 1)):
            # Small inputs ride the SP ring behind the first chunks; the
            # nd-derived factors then overlap the rest of the stream.
            nc.sync.dma_start(rw_t[:], rw_d.ap())
            nc.sync.dma_start(dn_t[:], dn_d.ap())
            if not nocompute:
                nc.vector.tensor_copy(ndf[:], dn_t[:])
                nc.scalar.activation(g[:], ndf[:], copyf, bias=GAMMA, scale=-GAMMA)
                nc.scalar.activation(
                    coef[:], ndf[:], copyf,
                    bias=GAMMA * LMBDA, scale=-GAMMA * LMBDA,
                )
                # gb = b*g - b (reuses ndf storage); rwb = rw + gb in place.
                nc.scalar.activation(
                    ndf[:], g[:], identf, bias=nb_t[:, 0:1], scale=b_t[:, 0:1]
                )
                nc.vector.tensor_tensor(out=rw_t[:], in0=rw_t[:], in1=ndf[:], op=add)
                # pfx[t'] = prod_{s>=t'} coef_lo[s]
                nc.vector.tensor_tensor_scan(
                    out=pfx[:, ::-1],
                    data0=coef[lo, ::-1],
                    data1=zeros_lo[:],
                    initial=1.0,
                    op0=mult,
                    op1=add,
                )
        if not nocompute:
            for gi in range(ngrp_c):
                gg = j * ngrp_c + gi  # global psum-group index
                si = gg % stage_grps
                if si == 0:
                    st = spool.tile([2, sg_cols], F32)
                pt = qpool.tile([2, grp_cols], F32)
                for k in range(grp_mm):
                    cs = slice(
                        (gi * grp_mm + k) * MM_N, (gi * grp_mm + k + 1) * MM_N
                    )
                    nc.tensor.matmul(
                        pt[:, k * MM_N : (k + 1) * MM_N],
                        lhsT=w2_t[:],
                        rhs=ct[:, cs],
                        start=True,
                        stop=True,
                    )
                if gg % 5 < 3:
                    nc.scalar.activation(
                        st[:, si * grp_cols : (si + 1) * grp_cols], pt[:], copyf
                    )
                else:
                    nc.vector.tensor_copy(
                        st[:, si * grp_cols : (si + 1) * grp_cols], pt[:]
                    )
                if si == stage_grps - 1:
                    # Scatter staged rows into scan layout (Pool SWDGE: waits
                    # on the evacuation park the idle Pool queue only).
                    # Streaming order is hi-half first: staged trajectory-half
                    # index thn maps to partition 64+thn (hi) / thn-64 (lo).
                    thn = ((gg + 1 - stage_grps) * grp_cols) // tp
                    npart = sg_cols // tp
                    dst = BL + thn if thn < BL else thn - BL
                    nc.gpsimd.dma_start(
                        v_raw[dst : dst + npart, :], st[0:1, :]
                    )
                    nc.gpsimd.dma_start(
                        nv_raw[dst : dst + npart, :], st[1:2, :]
                    )
            if j == nchunk // 2 - 1:
                # hi half fully scattered (once those DMAs land): run its
                # td0 chain + reverse scan in-stream on the idle DVE.
                td0_half(hi)
                scan_half(hi)
                nc.vector.scalar_tensor_tensor(
                    out=tgt[hi, :], in0=v_raw[hi, :], scalar=b_t[hi, 0:1],
                    in1=adv[hi, :], op0=add, op1=add,
                )
            if j == min(nchunk // 2 + 1, nchunk - 1):
                # Boundary + hi outputs, emitted a couple chunks later so
                # their waits are already satisfied when Pool reaches them.
                nc.gpsimd.dma_start(bnd[:], adv[hi, 0:1])
                nc.gpsimd.dma_start(adv_d.ap()[hi, :], adv[hi, :])
                nc.gpsimd.dma_start(tgt_d.ap()[hi, :], tgt[hi, :])

    if nocompute:
        # still write outputs so the IO footprint matches (garbage values;
        # rw_t is used because it is actually written by a DMA above)
        nc.gpsimd.dma_start(adv_d.ap(), rw_t[:])
        nc.gpsimd.dma_start(tgt_d.ap(), rw_t[:])
        return

    # Post-stream tail: lo-half td0 chain, zero-seeded scan, boundary patch
    # via the in-stream suffix cumprod, fused target, outputs.
    td0_half(lo)
    scan_half(lo)
    nc.vector.scalar_tensor_tensor(
        out=adv[lo, :], in0=pfx[:], scalar=bnd[:, 0:1], in1=adv[lo, :],
        op0=mult, op1=add,
    )
    nc.gpsimd.dma_start(adv_d.ap()[lo, :], adv[lo, :])
    # value_target = adv + value = (v_raw + b) + adv, fused.
    nc.vector.scalar_tensor_tensor(
        out=tgt[lo, :], in0=v_raw[lo, :], scalar=b_t[lo, 0:1], in1=adv[lo, :],
        op0=add, op1=add,
    )
    nc.gpsimd.dma_start(tgt_d.ap()[lo, :], tgt[lo, :])


def build_program(
    t_total=T, nchunk=None, repeat=1, nocompute=False, bufs=3,
    bench_internal=False, chunk_cols=8192, sbufs=4, grp_mm=GRP_MM,
    stage_grps=STAGE_GRPS, psum_bufs=None,
):
    """Build the per-core Bass program (all 8 cores run it SPMD on their own
    shard). obs/nobs DRAM layout is [D, (h, b, t')] (host pre-transposed);
    rw/dn/adv/tgt are [(h, b), t'] as in the baseline. repeat>1 re-runs the
    whole pipeline inside one NEFF (test.py uses the delta vs repeat=1 to
    measure per-iteration HW time). bench_internal makes obs/next_obs
    Internal DRAM (not shipped per call; garbage values) so benchmark calls
    are cheap — timing-only builds."""
    tp = t_total // H  # timesteps per partition
    ncols = BL * t_total  # matmul columns
    assert ncols % chunk_cols == 0
    nchunk = ncols // chunk_cols
    assert chunk_cols % (grp_mm * MM_N) == 0
    assert (stage_grps * grp_mm * MM_N) % tp == 0
    if psum_bufs is None:
        psum_bufs = 8 // grp_mm

    nc = bacc.Bacc(
        "TRN2", target_bir_lowering=False, debug=False, enable_asserts=False
    )

    big_kind = "Internal" if bench_internal else "ExternalInput"
    onz_d = nc.dram_tensor("onz", [P, ncols], BF16, kind=big_kind)
    rw_d = nc.dram_tensor("rw", [P, tp], F32, kind="ExternalInput")
    dn_d = nc.dram_tensor("dn", [P, tp], U8, kind="ExternalInput")
    w2_d = nc.dram_tensor("w2", [P, 2], BF16, kind="ExternalInput")
    b_d = nc.dram_tensor("b", [1], F32, kind="ExternalInput")
    adv_d = nc.dram_tensor("adv", [P, tp], F32, kind="ExternalOutput")
    tgt_d = nc.dram_tensor("tgt", [P, tp], F32, kind="ExternalOutput")

    with tile.TileContext(nc) as tc, ExitStack() as ctx:
        cpool = ctx.enter_context(tc.tile_pool(name="const", bufs=1))
        kpool = ctx.enter_context(tc.tile_pool(name="chunks", bufs=bufs))
        ppool = ctx.enter_context(tc.tile_pool(name="pers", bufs=1))
        dpool = ctx.enter_context(tc.tile_pool(name="dbl", bufs=2))
        spool = ctx.enter_context(tc.tile_pool(name="stage", bufs=sbufs))
        qpool = ctx.enter_context(tc.psum_pool(name="psum", bufs=psum_bufs))

        # Masked value-head weight pair: col 0 selects obs (parts 0..63),
        # col 1 selects next_obs (parts 64..127).
        w2_t = cpool.tile([P, 2], BF16)
        nc.sync.dma_start(w2_t[:], w2_d.ap())
        b_t = cpool.tile([P, 1], F32)
        nc.sync.dma_start(b_t[:], b_d.ap().unsqueeze(0).broadcast_to([P, 1]))
        nb_t = cpool.tile([P, 1], F32)  # -b, bias operand for gb
        nc.scalar.activation(
            nb_t[:], b_t[:], mybir.ActivationFunctionType.Copy,
            bias=0.0, scale=-1.0,
        )
        zeros_lo = cpool.tile([BL, tp], F32)
        nc.vector.memset(zeros_lo[:], 0.0)
        bnd = cpool.tile([BL, 1], F32)
        consts = (w2_t, b_t, nb_t, zeros_lo, bnd)

        for _rep in range(repeat):
            _build_iter(
                nc, kpool, ppool, dpool, spool, qpool, consts,
                onz_d, rw_d, dn_d, adv_d, tgt_d, tp, chunk_cols,
                nchunk, nocompute=nocompute, grp_mm=grp_mm,
                stage_grps=stage_grps,
            )

    # Runs the bacc pipeline (register allocation etc.) — required before
    # serializing for the walrus compiler.
    nc.finalize()
    return nc


_NC_CACHE = None


def _get_nc():
    global _NC_CACHE
    if _NC_CACHE is None:
        _NC_CACHE = build_program()
    return _NC_CACHE


def _hmajor(x, tp_cols):
    """[BL, H*tp_cols] row-major -> [H*BL, tp_cols] with row p = h*BL + b."""
    return np.ascontiguousarray(
        x.reshape(BL, H, tp_cols).transpose(1, 0, 2).reshape(H * BL, tp_cols)
    )


def _unhmajor(y):
    """Inverse of _hmajor for outputs: [H*BL, tp] -> [BL, H*tp]."""
    tp = y.shape[1]
    return y.reshape(H, BL, tp).transpose(1, 0, 2).reshape(BL, H * tp)


def _dmajor(x, tp):
    """[BL, T, D] bf16 -> [D, ncols] with col n = (1-h, b, t'): hi first."""
    return np.ascontiguousarray(
        x.reshape(BL, H, tp, D)[:, ::-1].transpose(3, 1, 0, 2).reshape(
            D, BL * H * tp
        )
    )


def shard_inputs(obs, next_obs, reward, done, W, b):
    """Split full inputs into the 8 per-core input maps."""
    obs = np.asarray(obs, dtype=np.float32).reshape(B, T, D)
    nobs = np.asarray(next_obs, dtype=np.float32).reshape(B, T, D)
    obs = obs.astype(ml_dtypes.bfloat16)
    nobs = nobs.astype(ml_dtypes.bfloat16)
    rw = np.asarray(reward, dtype=np.float32).reshape(B, T)
    dn = np.asarray(done).astype(np.uint8, copy=False).reshape(B, T)
    w_np = np.ascontiguousarray(np.asarray(W, dtype=np.float32)).reshape(D)
    b_np = np.ascontiguousarray(np.asarray(b, dtype=np.float32)).reshape(1)

    w2 = np.zeros((P, 2), ml_dtypes.bfloat16)
    w2[0:BL, 0] = w_np
    w2[BL:P, 1] = w_np

    tp = T // H
    in_maps = []
    for i in range(NCORES):
        sl = slice(i * BL, (i + 1) * BL)
        in_maps.append(
            {
                "onz": np.concatenate(
                    [_dmajor(obs[sl], tp), _dmajor(nobs[sl], tp)], axis=0
                ),
                "rw": _hmajor(rw[sl], tp),
                "dn": _hmajor(dn[sl], tp),
                "w2": w2,
                "b": b_np,
            }
        )
    return in_maps


def gather_outputs(results):
    advantage = np.concatenate(
        [_unhmajor(r["adv"]) for r in results], axis=0
    ).reshape(B, T, 1)
    value_target = np.concatenate(
        [_unhmajor(r["tgt"]) for r in results], axis=0
    ).reshape(B, T, 1)
    return advantage, value_target


def kernel(obs, next_obs, reward, done, W, b):
    global LAST_RESULTS
    nc = _get_nc()
    in_maps = shard_inputs(obs, next_obs, reward, done, W, b)
    res = run_bass_kernel_spmd(nc, in_maps, core_ids=list(range(NCORES)))
    LAST_RESULTS = res
    return gather_outputs(res.results)
